# revision 1
# baseline (speedup 1.0000x reference)
"""Trainium2 Bass kernel for nn_AttentionTail.

Reference (B=2, N=300, C=256, H=2 heads, hd=128, L=21760):
  q = query @ Wq.T ; k = key @ Wk.T   (2 heads of 128)
  attn[b,n,l,h] = scale * <q_h, k_h>
  per level i (128^2, 64^2, 32^2, 16^2 keys): z = relu(attn_i @ Wl[i].T + bl[i])
  bilinear-upsample each level map to 128x128, concat channels,
  mask = relu(concat @ Wf.T + bf)

Host folds Wq, Wk, Wl, Wf, scale into 8 per-(level,channel) query vectors:
  qhat_{i,c} = scale*Wf[0,2i+c] * [Wl[i][c,0]*qp[:,:128] | Wl[i][c,1]*qp[:,128:]] @ Wk
  v_{i,c}[n,l] = <qhat_{i,c}[n], key[l]> ;  b_{i,c} = Wf[0,2i+c]*bl[i][c]
  contribution = relu(v+b) if Wf>0 else min(v+b, 0)   (= Wf * relu(attn-path))
  y_i = contrib_0 + contrib_1
  mask = relu(y_0 + sum_{i>=1} Ur_i @ Y_i @ Uc_i.T + bf)

Device (8 cores SPMD, B x N-quarter sharding, 75 queries/core):
  stream keyT (host-transposed, levels 1-3 in (w,h) column order), fp32r
  matmuls for scores, ACT/DVE relu+combine, PE-matmul separable upsample,
  PE transposes for layout, final relu, transposed store.
"""

import sys
import numpy as np

sys.path.insert(0, "/opt/trn_rl_repo")

import concourse.bass as bass
import concourse.bacc as bacc
import concourse.tile as tile
from concourse import mybir
from concourse.bass_utils import run_bass_kernel_spmd

F32 = mybir.dt.float32
F32R = mybir.dt.float32r
BF16 = mybir.dt.bfloat16

B, N, C = 2, 300, 256
HD = 128
SCALE = HD ** -0.5
HW_LVL = ((128, 128), (64, 64), (32, 32), (16, 16))
L = 21760
L_DEV = 22016            # lvl3 padded h 16->32 on host (+256 zero cols)
LVL_OFF = [0, 16384, 20480, 21504]
NQ = 75
NCORES = 8

SPANS = ([(s * 512, 512, 0) for s in range(32)]
         + [(16384 + s * 512, 512, 1) for s in range(8)]
         + [(20480 + s * 512, 512, 2) for s in range(2)]
         + [(21504, 512, 3)])

LVL_H = [128, 64, 32, 32]   # partition rows per block (lvl3 zero-padded)
LVL_W = [128, 64, 32, 16]
WSUB = [1, 2, 4, 4]          # w-columns per 128-key block
WBASE = [0, 0, 64, 96]       # row base of each level inside zcat / uc


def interp_matrix(src, dst):
    U = np.zeros((dst, src), np.float32)
    s = src / dst
    for d in range(dst):
        x = (d + 0.5) * s - 0.5
        x0 = int(np.floor(x))
        fr = x - x0
        a, b = max(0, min(src - 1, x0)), max(0, min(src - 1, x0 + 1))
        U[d, a] += 1 - fr
        U[d, b] += fr
    return U


def _build_program(signs, bf_val, reps=1):
    nc = bacc.Bacc("TRN2", target_bir_lowering=False)
    keyT = nc.dram_tensor("keyT", [C, L_DEV], F32R, kind="ExternalInput")
    qhatT = nc.dram_tensor("qhatT", [C, 1024], F32R, kind="ExternalInput")
    bias_in = nc.dram_tensor("bias_in", [1, 9], F32, kind="ExternalInput")
    ur_in = nc.dram_tensor("ur_in", [64, 384], F32R, kind="ExternalInput")
    uc_in = nc.dram_tensor("uc_in", [112, 128], BF16, kind="ExternalInput")
    ident_in = nc.dram_tensor("ident_in", [128, 128], F32, kind="ExternalInput")
    identb_in = nc.dram_tensor("identb_in", [128, 128], BF16, kind="ExternalInput")
    out_d = nc.dram_tensor("out", [NQ, 128 * 128], F32, kind="ExternalOutput")

    with tile.TileContext(nc) as tc:
        for _ in range(reps):
            _body(nc, tc, keyT, qhatT, bias_in, ur_in, uc_in, ident_in,
                  identb_in, out_d, signs, bf_val)
    nc.compile()
    return nc


def _body(nc, tc, keyT, qhatT, bias_in, ur_in, uc_in, ident_in,
          identb, out_d, signs, bf_val):
    from contextlib import ExitStack
    ctx = ExitStack()
    with ctx:
        consts = ctx.enter_context(tc.tile_pool(name="consts", bufs=1))
        kpool = ctx.enter_context(tc.tile_pool(name="kpool", bufs=3))
        upool = ctx.enter_context(tc.tile_pool(name="upool", bufs=3))
        spool = ctx.enter_context(tc.tile_pool(name="spool", bufs=1))
        zpool = ctx.enter_context(tc.tile_pool(name="zpool", bufs=1))
        fpool = ctx.enter_context(tc.tile_pool(name="fpool", bufs=3))
        ps_attn = ctx.enter_context(tc.tile_pool(name="ps_attn", bufs=2, space="PSUM"))
        ps_up = ctx.enter_context(tc.tile_pool(name="ps_up", bufs=2, space="PSUM"))
        ps_tr = ctx.enter_context(tc.tile_pool(name="ps_tr", bufs=2, space="PSUM"))

        # ---- constants ----
        qh0 = consts.tile([128, 1024], F32R, name="qh0")
        qh1 = consts.tile([128, 1024], F32R, name="qh1")
        nc.sync.dma_start(out=qh0, in_=qhatT[0:128, :])
        nc.sync.dma_start(out=qh1, in_=qhatT[128:256, :])
        ur = consts.tile([64, 384], F32R, name="ur")
        nc.sync.dma_start(out=ur, in_=ur_in[:, :])
        uc = consts.tile([112, 128], BF16, name="uc")
        nc.sync.dma_start(out=uc, in_=uc_in[:, :])
        ident = consts.tile([128, 128], F32, name="ident")
        nc.sync.dma_start(out=ident, in_=ident_in[:, :])
        identb_sb = consts.tile([128, 128], BF16, name="identb_sb")
        nc.sync.dma_start(out=identb_sb, in_=identb[:, :])
        bias_sb = consts.tile([128, 9], F32, name="bias_sb")
        nc.sync.dma_start(out=bias_sb, in_=bias_in[0:1, :].to_broadcast([128, 9]))

        # ---- level-map buffers ----
        y0 = spool.tile([128, NQ * 128], F32, name="y0")          # [c, (n, r)]
        sr_l = [None,
                spool.tile([64, 64 * NQ], F32R, name="s1r"),       # [h, (w, n)]
                spool.tile([32, 32 * NQ], F32R, name="s2r"),
                spool.tile([32, 16 * NQ], F32R, name="s3r")]

        # ---- attention over key spans ----
        for off, ln, lvl in SPANS:
            nblk = ln // 128
            k0 = kpool.tile([128, 512], F32R, tag="k0", name="k0")
            k1 = kpool.tile([128, 512], F32R, tag="k1", name="k1")
            nc.sync.dma_start(out=k0[:, :ln], in_=keyT[0:128, off:off + ln])
            nc.sync.dma_start(out=k1[:, :ln], in_=keyT[128:256, off:off + ln])
            ps = ps_attn.tile([128, 1024], F32, tag="ps", name="ps")
            for j in range(nblk):
                pslice = ps[:, j * 256:(j + 1) * 256]
                nc.tensor.matmul(pslice,
                                 k0[:, j * 128:(j + 1) * 128],
                                 qh0[:, lvl * 256:(lvl + 1) * 256],
                                 start=True, stop=False)
                nc.tensor.matmul(pslice,
                                 k1[:, j * 128:(j + 1) * 128],
                                 qh1[:, lvl * 256:(lvl + 1) * 256],
                                 start=False, stop=True)

            u0 = upool.tile([128, 300], F32, tag="u0", name="u0")
            u1 = upool.tile([128, 300], F32, tag="u1", name="u1")
            psv = ps.rearrange("p (j x) -> p j x", x=256)
            for ch, ut in ((0, u0), (1, u1)):
                src = psv[:, 0:nblk, ch * NQ:(ch + 1) * NQ]
                dst = ut.rearrange("p (j n) -> p j n", n=NQ)[:, 0:nblk, :]
                bval = bias_sb[:, lvl * 2 + ch:lvl * 2 + ch + 1]
                if signs[lvl][ch] > 0:
                    nc.scalar.activation(dst, src,
                                         mybir.ActivationFunctionType.Relu,
                                         bias=bval, scale=1.0)
                else:
                    nc.vector.tensor_scalar(dst, src, bval, 0.0,
                                            mybir.AluOpType.add,
                                            mybir.AluOpType.min)

            if lvl == 0:
                r0 = off // 128
                dstv = y0.rearrange("p (n r) -> p n r", r=128)[:, :, r0:r0 + nblk]
                u0v = u0.rearrange("p (j n) -> p j n", n=NQ)[:, 0:nblk, :].transpose([0, 2, 1])
                u1v = u1.rearrange("p (j n) -> p j n", n=NQ)[:, 0:nblk, :].transpose([0, 2, 1])
                nc.vector.tensor_tensor(dstv, u0v, u1v, mybir.AluOpType.add)
            else:
                h, ws = LVL_H[lvl], WSUB[lvl]
                jb0 = (off - LVL_OFF[lvl]) // 128
                for j in range(nblk):
                    for wsub in range(ws):
                        w = (jb0 + j) * ws + wsub
                        pa = wsub * h
                        nc.vector.tensor_tensor(
                            sr_l[lvl][:, w * NQ:(w + 1) * NQ],
                            u0[pa:pa + h, j * NQ:(j + 1) * NQ],
                            u1[pa:pa + h, j * NQ:(j + 1) * NQ],
                            mybir.AluOpType.add)

        # ---- step A: Z_l[r, (w, n)] = Ur_l.T @ s_lr ----
        zs = [None,
              zpool.tile([128, 64 * NQ], BF16, name="z1"),
              zpool.tile([128, 32 * NQ], BF16, name="z2"),
              zpool.tile([128, 16 * NQ], BF16, name="z3")]
        for lvl in (1, 2, 3):
            h = LVL_H[lvl]
            tot = LVL_W[lvl] * NQ
            lhs = ur[0:h, (lvl - 1) * 128:lvl * 128]
            for ci in range((tot + 479) // 480):
                c0 = ci * 480
                cn = min(480, tot - c0)
                zp = ps_up.tile([128, 512], F32, tag="zp", name="zp")
                nc.tensor.matmul(zp[:, :cn], lhs,
                                 sr_l[lvl][:, c0:c0 + cn],
                                 start=True, stop=True)
                nc.vector.tensor_copy(zs[lvl][:, c0:c0 + cn], zp[:, :cn])

        # ---- transpose Z per query (batched 4) -> zcat[w_cat, (n, r)] bf16 ----
        zcat = zpool.tile([112, NQ * 128], BF16, name="zcat")
        for lvl in (1, 2, 3):
            w = LVL_W[lvl]
            zv = zs[lvl].rearrange("p (w n) -> p w n", n=NQ)
            for g in range((NQ + 3) // 4):
                nsz = min(4, NQ - g * 4)
                pt = ps_tr.tile([128, 512], BF16, tag="pt", name="pt")
                for k in range(nsz):
                    n = g * 4 + k
                    nc.tensor.matmul(pt[0:w, k * 128:(k + 1) * 128],
                                     zv[:, :, n],
                                     identb_sb,
                                     is_transpose=True)
                nc.vector.tensor_copy(
                    zcat[WBASE[lvl]:WBASE[lvl] + w,
                         g * 512:g * 512 + nsz * 128],
                    pt[0:w, 0:nsz * 128])

        # ---- step B + level-0 add + final relu + transpose + store ----
        for g in range((NQ + 3) // 4):
            nsz = min(4, NQ - g * 4)
            nn = nsz * 128
            pb = ps_up.tile([128, 512], F32, tag="zp", name="pb")
            nc.tensor.matmul(pb[:, :nn], uc[:, :],
                             zcat[:, g * 512:g * 512 + nn],
                             start=True, stop=True)
            fin = fpool.tile([128, 512], F32, tag="fin", name="fin")
            nc.vector.tensor_tensor(fin[:, :nn], pb[:, :nn],
                                    y0[:, g * 512:g * 512 + nn],
                                    mybir.AluOpType.add)
            nc.scalar.activation(fin[:, :nn], fin[:, :nn],
                                 mybir.ActivationFunctionType.Relu,
                                 bias=bias_sb[:, 8:9], scale=1.0)
            pt2 = ps_tr.tile([128, 512], F32, tag="pt", name="pt2")
            for k in range(nsz):
                nc.tensor.matmul(pt2[:, k * 128:(k + 1) * 128],
                                 fin[:, k * 128:(k + 1) * 128],
                                 ident,
                                 is_transpose=True)
            outT = fpool.tile([128, 512], F32, tag="outT", name="outT")
            nc.vector.tensor_copy(outT[:, :nn], pt2[:, :nn])
            dram = out_d[g * 4:g * 4 + nsz, :].rearrange("n (r c) -> r n c", c=128)
            nc.sync.dma_start(out=dram,
                              in_=outT.rearrange("p (n c) -> p n c", c=128)[:, 0:nsz, :])


def _host_prep(query, key, Wq, Wk, Wl, bl, Wf, bf):
    query = np.asarray(query, np.float32)
    key = np.asarray(key, np.float32)
    Wq, Wk = np.asarray(Wq, np.float32), np.asarray(Wk, np.float32)
    Wl, bl = np.asarray(Wl, np.float32), np.asarray(bl, np.float32)
    Wf, bf = np.asarray(Wf, np.float32), np.asarray(bf, np.float32)

    qproj = query @ Wq.T
    qhat = np.zeros((4, 2, B, N, C), np.float32)
    biases = np.zeros((1, 9), np.float32)
    biases[0, 8] = float(bf[0])
    signs = [[1, 1] for _ in range(4)]
    for i in range(4):
        for c in range(2):
            wf = float(Wf[0, 2 * i + c])
            qt = np.concatenate([Wl[i][c, 0] * qproj[..., :HD],
                                 Wl[i][c, 1] * qproj[..., HD:]], -1)
            qhat[i, c] = (SCALE * wf) * (qt @ Wk)
            biases[0, i * 2 + c] = wf * bl[i][c]
            signs[i][c] = 1 if wf >= 0 else -1

    keyTs = []
    for b in range(B):
        cols = [key[b, :16384]]
        for i in (1, 2):
            h, w = HW_LVL[i]
            blk = key[b, LVL_OFF[i]:LVL_OFF[i] + h * w].reshape(h, w, C)
            cols.append(np.ascontiguousarray(blk.transpose(1, 0, 2)).reshape(-1, C))
        blk3 = key[b, LVL_OFF[3]:LVL_OFF[3] + 256].reshape(16, 16, C)
        blk3 = np.concatenate([blk3, np.zeros((16, 16, C), np.float32)], 0)  # h pad
        cols.append(np.ascontiguousarray(blk3.transpose(1, 0, 2)).reshape(-1, C))
        kb = np.concatenate(cols, 0)
        keyTs.append(np.ascontiguousarray(kb.T))

    ur_in = np.zeros((64, 384), np.float32)
    uc_in = np.zeros((112, 128), np.float32)
    for i in (1, 2, 3):
        h, w = HW_LVL[i]
        ur_in[0:h, (i - 1) * 128:i * 128] = interp_matrix(h, 128).T
        uc_in[WBASE[i]:WBASE[i] + w, :] = interp_matrix(w, 128).T

    import ml_dtypes
    uc_bf = uc_in.astype(ml_dtypes.bfloat16)
    ident = np.eye(128, dtype=np.float32)
    identb = ident.astype(ml_dtypes.bfloat16)

    in_maps = []
    for core in range(NCORES):
        b, q0 = core // 4, (core % 4) * NQ
        qh = np.zeros((C, 1024), np.float32)
        for i in range(4):
            for c in range(2):
                qh[:, i * 256 + c * NQ:i * 256 + (c + 1) * NQ] = \
                    qhat[i, c, b, q0:q0 + NQ].T
        in_maps.append({
            "keyT": keyTs[b],
            "qhatT": qh,
            "bias_in": biases,
            "ur_in": ur_in,
            "uc_in": uc_bf,
            "ident_in": ident,
            "identb_in": identb,
        })
    return in_maps, signs, float(bf[0])


def kernel(query, key, Wq, Wk, Wl, bl, Wf, bf, hw_lvl=None, trace=False, reps=1):
    in_maps, signs, bf_val = _host_prep(query, key, Wq, Wk, Wl, bl, Wf, bf)
    nc = _build_program(signs, bf_val, reps=reps)
    res = run_bass_kernel_spmd(nc, in_maps, list(range(NCORES)), trace=trace)
    out = np.zeros((B, N, 128 * 128, 1), np.float32)
    for core in range(NCORES):
        b, q0 = core // 4, (core % 4) * NQ
        out[b, q0:q0 + NQ, :, 0] = res.results[core]["out"]
    kernel.last_results = res
    return out


kernel.last_results = None

